# revision 1
# baseline (speedup 1.0000x reference)
"""Trainium2 Bass kernel for nn_EmberBlock (dense transformer block with LIF-gated
attention). 8-core SPMD: head-parallel attention (4 heads/core, one batch per
4-core group) + sequence-parallel MLP after a 4-rank ReduceScatter.

kernel(**inputs) takes FULL unsharded inputs (as in reference.setup_inputs())
and returns the FULL [B, T, C] output.
"""
import numpy as np
import ml_dtypes

import concourse.bass as bass
import concourse.mybir as mybir
import concourse.tile as tile
from concourse import bacc
from concourse.bass_utils import run_bass_kernel_spmd

F32 = mybir.dt.float32
BF16 = mybir.dt.bfloat16
AF = mybir.ActivationFunctionType
ALU = mybir.AluOpType

# model dims (hardcoded per spec)
B, T, C = 2, 2048, 1024
H, D = 16, 64
FF = 4 * C                    # 4096
N_CORES = 8
GROUP = 4                     # cores per batch
HL = H // GROUP               # 4 local heads
LC = HL * D                   # 256 local head feature cols
EPS_LN = 1e-5
P = 128                       # partitions
NT = T // P                   # 16 token tiles per batch
NQS = 4                       # q-slabs of 512 tokens
MLP_TOK = 512                 # tokens per core in MLP phase (4 tiles)

_CACHED_NC = None


def _build(phases="all"):
    nc = bacc.Bacc(None, target_bir_lowering=False, debug=False, num_devices=N_CORES)

    # ---------------- I/O ----------------
    x_b = nc.dram_tensor("x_b", [T, C], F32, kind="ExternalInput")
    x_res = nc.dram_tensor("x_res", [MLP_TOK, C], F32, kind="ExternalInput")
    wqkv = nc.dram_tensor("wqkv", [C, 3 * LC], BF16, kind="ExternalInput")
    bqkv = nc.dram_tensor("bqkv", [3 * LC], F32, kind="ExternalInput")
    wproj = nc.dram_tensor("wproj", [LC, C], BF16, kind="ExternalInput")
    bproj = nc.dram_tensor("bproj", [C], F32, kind="ExternalInput")
    wfc = nc.dram_tensor("wfc", [C, FF], BF16, kind="ExternalInput")
    bfc = nc.dram_tensor("bfc", [FF], F32, kind="ExternalInput")
    wmlp = nc.dram_tensor("wmlp", [FF, C], BF16, kind="ExternalInput")
    bmlp = nc.dram_tensor("bmlp", [C], F32, kind="ExternalInput")
    lif = nc.dram_tensor("lif", [4, HL], F32, kind="ExternalInput")
    out = nc.dram_tensor("out", [MLP_TOK, C], F32, kind="ExternalOutput")

    # RS bounce buffers (internal DRAM)
    rs_in = nc.dram_tensor("rs_in", [T, C], F32)
    rs_out = nc.dram_tensor("rs_out", [MLP_TOK, C], F32)

    # constants embedded in the NEFF
    id_bf = nc.inline_tensor(np.eye(P, dtype=ml_dtypes.bfloat16), name="id_bf")
    tri_np = np.tril(np.ones((P, P), np.float32)).astype(ml_dtypes.bfloat16)
    tri_c = nc.inline_tensor(tri_np, name="tri_c")

    replica_groups = [[0, 1, 2, 3], [4, 5, 6, 7]]

    from contextlib import ExitStack
    with tile.TileContext(nc) as tc, ExitStack() as root_ctx:
        attn_ctx = ExitStack()
        consts = root_ctx.enter_context(tc.tile_pool(name="consts", bufs=1))
        zero_c = consts.tile([P, 1], F32)
        nc.vector.memset(zero_c[:], 0.0)
        nc.const_aps.aps[(F32, 0.0)] = zero_c[:]
        eps_c = consts.tile([P, 1], F32)
        nc.vector.memset(eps_c[:], EPS_LN)
        nc.const_aps.aps[(F32, EPS_LN)] = eps_c[:]
        ident = consts.tile([P, P], BF16)
        nc.sync.dma_start(out=ident[:], in_=id_bf[:, :])
        tri = consts.tile([P, P], BF16)
        nc.sync.dma_start(out=tri[:], in_=tri_c[:, :])
        # per-head LIF constants broadcast to all partitions: [128, 4, HL]
        lif_sb = consts.tile([P, 4, HL], F32)
        nc.sync.dma_start(out=lif_sb[:], in_=lif[None, :, :].to_broadcast((P, 4, HL)))
        # biases in per-partition layout
        bqkv_sb = consts.tile([P, 6], F32)
        nc.sync.dma_start(out=bqkv_sb[:], in_=bqkv.rearrange("(t p) -> p t", p=P))
        bfc_sb = consts.tile([P, FF // P], F32)
        nc.sync.dma_start(out=bfc_sb[:], in_=bfc.rearrange("(t p) -> p t", p=P))
        # free-dim biases broadcast across partitions
        bproj_sb = consts.tile([P, C], BF16)
        nc.gpsimd.dma_start(out=bproj_sb[:], in_=bproj[None, :].to_broadcast((P, C)))
        bmlp_sb = consts.tile([P, C], BF16)
        nc.gpsimd.dma_start(out=bmlp_sb[:], in_=bmlp[None, :].to_broadcast((P, C)))
        # attention-projection weights (2 k-tiles), small -> consts
        wproj_sb = consts.tile([P, 2, C], BF16)
        for kt in range(2):
            nc.sync.dma_start(out=wproj_sb[:, kt, :], in_=wproj[kt * P:(kt + 1) * P, :])

        # =========== Phase 1: LN1 over all T tokens + h^T ===========
        xpool = root_ctx.enter_context(tc.tile_pool(name="xpool", bufs=2))
        stat_pool = root_ctx.enter_context(tc.tile_pool(name="stats", bufs=4))
        tp_psum = root_ctx.enter_context(tc.tile_pool(name="tp_psum", bufs=2, space="PSUM"))
        mm_psum = root_ctx.enter_context(tc.tile_pool(name="mm_psum", bufs=2, space="PSUM"))
        smallp = root_ctx.enter_context(tc.tile_pool(name="smallp", bufs=10))
        p12 = attn_ctx.enter_context(tc.tile_pool(name="p12", bufs=1))

        hT = p12.tile([P, C // P, T], BF16)       # [128, 8, 2048] feature-major h
        wqkv_sb = p12.tile([P, 8, 3 * LC], BF16)  # 8 k-tiles of wqkv
        for kt in range(8):
            nc.sync.dma_start(out=wqkv_sb[:, kt, :], in_=wqkv[kt * P:(kt + 1) * P, :])

        def layernorm_tile(x_tile, h_out, tag):
            """x_tile [128, C] f32 -> h_out [128, C] bf16 (normalized, no affine)."""
            stats = stat_pool.tile([P, 2, 6], F32, name=f"st_{tag}")
            nc.vector.bn_stats(out=stats[:, 0, :], in_=x_tile[:, 0:512])
            nc.vector.bn_stats(out=stats[:, 1, :], in_=x_tile[:, 512:1024])
            mv = stat_pool.tile([P, 2], F32, name=f"mv_{tag}")
            nc.vector.bn_aggr(out=mv[:], in_=stats[:])
            std = stat_pool.tile([P, 1], F32, name=f"sd_{tag}")
            nc.scalar.activation(std[:], mv[:, 1:2], AF.Sqrt, bias=EPS_LN)
            rstd = stat_pool.tile([P, 1], F32, name=f"rs_{tag}")
            nc.vector.reciprocal(rstd[:], std[:])
            nmr = stat_pool.tile([P, 1], F32, name=f"nm_{tag}")
            nc.vector.tensor_scalar(out=nmr[:], in0=mv[:, 0:1], scalar1=rstd[:],
                                    scalar2=-1.0, op0=ALU.mult, op1=ALU.mult)
            nc.scalar.activation(h_out, x_tile, AF.Identity, bias=nmr[:], scale=rstd[:])

        for tt in range(NT):
            x_tile = xpool.tile([P, C], F32, name="x_t", tag="x_t")
            nc.sync.dma_start(out=x_tile[:], in_=x_b[tt * P:(tt + 1) * P, :])
            h_tile = xpool.tile([P, C], BF16, name="h_t", tag="h_t")
            layernorm_tile(x_tile[:], h_tile[:], f"ln1_{tt}")
            # transpose h [128, C] -> hT[:, ft, tt*128:...]
            for fg in range(2):  # groups of 4 feature tiles -> one psum bank
                tp = tp_psum.tile([P, 4, P], BF16, name="htp", tag="htp")
                for j in range(4):
                    ft = fg * 4 + j
                    nc.tensor.transpose(tp[:, j, :], h_tile[:, ft * P:(ft + 1) * P],
                                        ident[:])
                nc.scalar.activation(hT[:, fg * 4:(fg + 1) * 4, tt * P:(tt + 1) * P],
                                     tp[:], AF.Copy)

        # =========== Phase 2: QKV^T = wqkv^T @ h (feature-major) ===========
        attn = attn_ctx.enter_context(tc.tile_pool(name="attn", bufs=1))

        qkvT = attn.tile([P, 6, T], BF16)   # rows: q(2 tiles) k(2) v(2)
        for mt in range(6):
            for ns in range(4):
                ps = mm_psum.tile([P, 512], F32, name="qkv_ps", tag="qkv_ps")
                for kt in range(8):
                    nc.tensor.matmul(ps[:],
                                     wqkv_sb[:, kt, mt * P:(mt + 1) * P],
                                     hT[:, kt, ns * 512:(ns + 1) * 512],
                                     start=(kt == 0), stop=(kt == 7))
                nc.scalar.activation(qkvT[:, mt, ns * 512:(ns + 1) * 512], ps[:],
                                     AF.Identity, bias=bqkv_sb[:, mt:mt + 1])

        # =========== Phase 3: V^T -> V (token-major) ===========
        v_tok = attn.tile([P, NT, LC], BF16)   # [128, 16, 256]
        for vt in range(2):
            for tg in range(4):  # 4 token tiles per psum bank
                tp = tp_psum.tile([P, 4, P], BF16, name="vtp", tag="htp")
                for j in range(4):
                    tt = tg * 4 + j
                    nc.tensor.transpose(tp[:, j, :],
                                        qkvT[:, 4 + vt, tt * P:(tt + 1) * P],
                                        ident[:])
                nc.scalar.activation(v_tok[:, tg * 4:(tg + 1) * 4, vt * P:(vt + 1) * P],
                                     tp[:], AF.Copy)

        # =========== Phase 4: attention per q-slab, per head ===========
        epool = attn_ctx.enter_context(tc.tile_pool(name="epool", bufs=2))
        e4pool = attn_ctx.enter_context(tc.tile_pool(name="e4pool", bufs=9))
        spool = attn_ctx.enter_context(tc.tile_pool(name="spool", bufs=3, space="PSUM"))
        mtpool = attn_ctx.enter_context(tc.tile_pool(name="mtpool", bufs=1))
        ypool = attn_ctx.enter_context(tc.tile_pool(name="ypool", bufs=2))
        yps_pool = attn_ctx.enter_context(tc.tile_pool(name="yps", bufs=1, space="PSUM"))
        o2pool = attn_ctx.enter_context(tc.tile_pool(name="o2pool", bufs=2))

        for qs in range(NQS):
            yT = [ypool.tile([P, 512], BF16, name=f"yT{i}_{qs}", tag=f"yT{i}")
                  for i in range(2)]  # [2 tiles of 128 feats][512 q] local Y^T
            state = {}

            def pass1(h):
                qrow = (h % 2) * D
                qtile_idx = h // 2
                ktile_idx = 2 + h // 2
                sm_cols = smallp.tile([P, 4], F32, name="smc", tag="smc")
                rsm_cols = smallp.tile([P, 4], F32, name="rsmc", tag="rsmc")
                e_ts, sprhs = {}, {}
                for qt in range(qs * 4, qs * 4 + 4):
                    j = qt - qs * 4
                    W = (qt + 1) * P
                    e_t = e4pool.tile([P, T], BF16, name="e_t", tag="e_t")
                    e_ts[j] = e_t
                    se_parts = smallp.tile([P, 8], F32, name="sep", tag="sep")
                    nsl = (W + 511) // 512
                    se_n = 0
                    for ks in range(nsl):
                        w0 = ks * 512
                        w1 = min(w0 + 512, W)
                        sl = w1 - w0
                        ps = spool.tile([P, 512], F32, name="s_ps", tag="s_ps")
                        nc.tensor.matmul(
                            ps[:, :sl],
                            qkvT[qrow:qrow + D, qtile_idx, qt * P:(qt + 1) * P],
                            qkvT[qrow:qrow + D, ktile_idx, w0:w1],
                            start=True, stop=True)
                        if w1 == W:
                            if sl > P:
                                nc.scalar.activation(
                                    e_t[:, w0:W - P], ps[:, :sl - P], AF.Exp,
                                    accum_out=se_parts[:, se_n:se_n + 1])
                                se_n += 1
                            nc.scalar.activation(e_t[:, W - P:W], ps[:, sl - P:sl],
                                                 AF.Exp)
                            nc.vector.tensor_tensor(out=e_t[:, W - P:W],
                                                    in0=e_t[:, W - P:W], in1=tri[:],
                                                    op=ALU.mult)
                            nc.vector.reduce_sum(se_parts[:, se_n:se_n + 1],
                                                 e_t[:, W - P:W],
                                                 axis=mybir.AxisListType.X)
                            se_n += 1
                        else:
                            nc.scalar.activation(
                                e_t[:, w0:w1], ps[:], AF.Exp,
                                accum_out=se_parts[:, se_n:se_n + 1])
                            se_n += 1
                    se = smallp.tile([P, 1], F32, name="se", tag="se")
                    nc.vector.reduce_sum(se[:], se_parts[:, :se_n],
                                         axis=mybir.AxisListType.X)
                    rp = smallp.tile([P, 1], F32, name="rp", tag="rp")
                    nc.vector.reciprocal(rp[:], se[:])
                    sprh = smallp.tile([P, 1], F32, name="sprh", tag="sprh")
                    nc.vector.tensor_scalar(out=sprh[:], in0=rp[:],
                                            scalar1=lif_sb[:, 0, h:h + 1],
                                            scalar2=None, op0=ALU.mult)
                    sprhs[j] = sprh
                state[h] = (e_ts, sprhs, sm_cols, rsm_cols)

            def pass2(h):
                qrow = (h % 2) * D
                e_ts, sprhs, sm_cols, rsm_cols = state.pop(h)
                mT = mtpool.tile([P, 16, 512], BF16, name="mT", tag="mT")
                for qt in range(qs * 4, qs * 4 + 4):
                    j = qt - qs * 4
                    W = (qt + 1) * P
                    e_t = e_ts[j]
                    sprh = sprhs[j]
                    f_t = epool.tile([P, T], BF16, name="f_t", tag="f_t")
                    nc.scalar.activation(f_t[:, :W], e_t[:, :W], AF.Tanh,
                                         bias=lif_sb[:, 1, h:h + 1], scale=sprh[:])
                    nc.vector.tensor_scalar(out=f_t[:, :W], in0=f_t[:, :W],
                                            scalar1=lif_sb[:, 2, h:h + 1],
                                            scalar2=lif_sb[:, 3, h:h + 1],
                                            op0=ALU.mult, op1=ALU.add)
                    m_t = epool.tile([P, T], BF16, name="m_t", tag="m_t")
                    nc.vector.tensor_tensor(out=m_t[:, :W], in0=f_t[:, :W],
                                            in1=e_t[:, :W], op=ALU.mult)
                    nc.vector.reduce_sum(sm_cols[:, j:j + 1], m_t[:, :W],
                                         axis=mybir.AxisListType.X)
                    nc.vector.reciprocal(rsm_cols[:, j:j + 1], sm_cols[:, j:j + 1])
                    nc.vector.tensor_scalar(out=m_t[:, :W], in0=m_t[:, :W],
                                            scalar1=rsm_cols[:, j:j + 1],
                                            scalar2=None, op0=ALU.mult)
                    for kg in range((qt + 1 + 3) // 4):
                        k0 = kg * 4
                        kn = min(4, qt + 1 - k0)
                        tp = tp_psum.tile([P, 4, P], BF16, name="mtp", tag="htp")
                        for kk in range(kn):
                            kb = k0 + kk
                            nc.tensor.transpose(tp[:, kk, :],
                                                m_t[:, kb * P:(kb + 1) * P], ident[:])
                        nc.scalar.activation(
                            mT[:, k0:k0 + kn, j * P:(j + 1) * P],
                            tp[:, :kn, :], AF.Copy)
                # PV
                yps = yps_pool.tile([D, 512], F32, name="yps", tag="yps")
                nkb = qs * 4 + 4
                for kb in range(nkb):
                    c0 = max(0, kb - qs * 4) * P
                    nc.tensor.matmul(yps[:, c0:512],
                                     v_tok[:, kb, h * D:(h + 1) * D],
                                     mT[:, kb, c0:512],
                                     start=(kb == 0), stop=(kb == nkb - 1))
                nc.scalar.activation(yT[h // 2][qrow:qrow + D, :], yps[:], AF.Copy)

            # two-deep cross-head pipeline: S/exp/stats of head h+1 are emitted
            # before the gate/transpose/PV chain of head h
            pass1(0)
            for h in range(1, HL):
                pass1(h)
                pass2(h - 1)
            pass2(HL - 1)
            # attn-proj for this q-slab: out2 = Y @ wproj  (token-major)
            for mt in range(4):
                o2 = o2pool.tile([P, C], F32, name="o2", tag="o2")
                for ns in range(2):
                    ps = mm_psum.tile([P, 512], F32, name="o2_ps", tag="qkv_ps")
                    for kt in range(2):
                        nc.tensor.matmul(ps[:],
                                         yT[kt][:, mt * P:(mt + 1) * P],
                                         wproj_sb[:, kt, ns * 512:(ns + 1) * 512],
                                         start=(kt == 0), stop=(kt == 1))
                    nc.scalar.activation(o2[:, ns * 512:(ns + 1) * 512], ps[:],
                                         AF.Copy)
                nc.sync.dma_start(
                    out=rs_in[qs * 512 + mt * P: qs * 512 + (mt + 1) * P, :],
                    in_=o2[:])

        # =========== ReduceScatter ===========
        if phases == "nors":
            for j in range(4):
                nc.sync.dma_start(out=rs_out[j * P:(j + 1) * P, :],
                                  in_=rs_in[j * P:(j + 1) * P, :])
        else:
            nc.gpsimd.collective_compute(
                "ReduceScatter", ALU.add, replica_groups=replica_groups,
                ins=[rs_in[:, :]], outs=[rs_out[:, :]])

        # release attention pools
        attn_ctx.close()

        # =========== Phase 5: MLP on 512 local tokens ===========
        mlp = root_ctx.enter_context(tc.tile_pool(name="mlp", bufs=1))
        wstream = root_ctx.enter_context(tc.tile_pool(name="wstream", bufs=1))

        wfc_sb = mlp.tile([P, 8, FF], BF16)
        for kt in range(8):
            nc.sync.dma_start(out=wfc_sb[:, kt, :], in_=wfc[kt * P:(kt + 1) * P, :])

        h2T = mlp.tile([P, 8, MLP_TOK], BF16)
        x1_t = []
        for j in range(4):
            rs_sb = xpool.tile([P, C], F32, name="rs_sb", tag="x_t")
            nc.sync.dma_start(out=rs_sb[:], in_=rs_out[j * P:(j + 1) * P, :])
            xr = xpool.tile([P, C], F32, name="xr", tag="x_t2")
            nc.sync.dma_start(out=xr[:], in_=x_res[j * P:(j + 1) * P, :])
            x1 = mlp.tile([P, C], F32, name=f"x1_{j}")
            nc.vector.tensor_tensor(out=x1[:], in0=xr[:], in1=rs_sb[:], op=ALU.add)
            nc.vector.tensor_tensor(out=x1[:], in0=x1[:], in1=bproj_sb[:], op=ALU.add)
            x1_t.append(x1)
            h2 = xpool.tile([P, C], BF16, name="h2", tag="h_t")
            layernorm_tile(x1[:], h2[:], f"ln2_{j}")
            for fg in range(2):
                tp = tp_psum.tile([P, 4, P], BF16, name="h2tp", tag="htp")
                for k in range(4):
                    ft = fg * 4 + k
                    nc.tensor.transpose(tp[:, k, :], h2[:, ft * P:(ft + 1) * P],
                                        ident[:])
                nc.scalar.activation(h2T[:, fg * 4:(fg + 1) * 4, j * P:(j + 1) * P],
                                     tp[:], AF.Copy)

        # FC + gelu: aT [128, 32, 512]
        aT = mlp.tile([P, FF // P, MLP_TOK], BF16)
        for mt in range(FF // P):
            ps = mm_psum.tile([P, 512], F32, name="fc_ps", tag="qkv_ps")
            for kt in range(8):
                nc.tensor.matmul(ps[:], wfc_sb[:, kt, mt * P:(mt + 1) * P],
                                 h2T[:, kt, :], start=(kt == 0), stop=(kt == 7))
            nc.scalar.activation(aT[:, mt, :], ps[:], AF.Gelu,
                                 bias=bfc_sb[:, mt:mt + 1])

        # MLP proj + residual: stream wmlp one 512-wide column half at a time
        o_sb_t = [xpool.tile([P, C], F32, name=f"o_sb{j}", tag="x_t2")
                  for j in range(2)]
        o_sb_t += [mlp.tile([P, C], F32, name=f"o_sb{j}") for j in range(2, 4)]
        for ns in range(2):
            wm_half = wstream.tile([P, FF // P, 512], BF16, name="wm_h", tag="wm_h")
            for kt in range(FF // P):
                nc.sync.dma_start(out=wm_half[:, kt, :],
                                  in_=wmlp[kt * P:(kt + 1) * P,
                                           ns * 512:(ns + 1) * 512])
            for j in range(4):
                ps = mm_psum.tile([P, 512], F32, name="o3_ps", tag="qkv_ps")
                for kt in range(FF // P):
                    nc.tensor.matmul(ps[:], aT[:, kt, j * P:(j + 1) * P],
                                     wm_half[:, kt, :],
                                     start=(kt == 0), stop=(kt == FF // P - 1))
                nc.vector.tensor_tensor(out=o_sb_t[j][:, ns * 512:(ns + 1) * 512],
                                        in0=x1_t[j][:, ns * 512:(ns + 1) * 512],
                                        in1=ps[:], op=ALU.add)
        for j in range(4):
            nc.vector.tensor_tensor(out=o_sb_t[j][:], in0=o_sb_t[j][:],
                                    in1=bmlp_sb[:], op=ALU.add)
            nc.sync.dma_start(out=out[j * P:(j + 1) * P, :], in_=o_sb_t[j][:])


    nc.compile()
    return nc


def _get_nc():
    global _CACHED_NC
    if _CACHED_NC is None:
        _CACHED_NC = _build()
    return _CACHED_NC


def _softplus(x):
    return np.log1p(np.exp(-np.abs(x))) + np.maximum(x, 0.0)


def _bf16(x):
    return np.ascontiguousarray(x.astype(ml_dtypes.bfloat16))


def kernel(x, ln1_w, ln1_b, w_attn, b_attn, w_attn_proj, b_attn_proj,
           threshold, leak, steepness, ln2_w, ln2_b,
           w_fc, b_fc, w_mlp_proj, b_mlp_proj):
    x = np.asarray(x, np.float32)
    f32 = lambda a: np.asarray(a, np.float32)
    ln1_w, ln1_b, w_attn, b_attn = map(f32, (ln1_w, ln1_b, w_attn, b_attn))
    w_attn_proj, b_attn_proj = f32(w_attn_proj), f32(b_attn_proj)
    threshold, leak, steepness = map(f32, (threshold, leak, steepness))
    ln2_w, ln2_b, w_fc, b_fc = map(f32, (ln2_w, ln2_b, w_fc, b_fc))
    w_mlp_proj, b_mlp_proj = f32(w_mlp_proj), f32(b_mlp_proj)

    # fold LN affine into the following matmuls (exact in fp32 algebra)
    wa = w_attn * ln1_w[:, None]
    ba = b_attn + ln1_b @ w_attn
    # fold 1/sqrt(D) into the q columns
    wa = wa.copy()
    wa[:, :C] *= 1.0 / np.sqrt(D)
    ba = ba.copy()
    ba[:C] *= 1.0 / np.sqrt(D)
    wf = w_fc * ln2_w[:, None]
    bf = b_fc + ln2_b @ w_fc

    # per-head LIF constants
    st = _softplus(steepness)
    lk = 1.0 / (1.0 + np.exp(-leak))
    th = np.abs(threshold) * 0.1

    wf_b = _bf16(wf)
    wm_b = _bf16(w_mlp_proj)

    in_maps = []
    for c in range(N_CORES):
        b = c // GROUP
        r = c % GROUP
        h0 = r * HL * D  # first local head feature col
        cols = (list(range(h0, h0 + LC))
                + list(range(C + h0, C + h0 + LC))
                + list(range(2 * C + h0, 2 * C + h0 + LC)))
        wqkv_local = _bf16(wa[:, cols])
        bqkv_local = np.ascontiguousarray(ba[cols], dtype=np.float32)
        wproj_local = _bf16(w_attn_proj[h0:h0 + LC, :])
        hsl = slice(r * HL, (r + 1) * HL)
        lif_local = np.stack([
            st[hsl] / 2.0,
            -(st[hsl] * th[hsl]) / 2.0,
            0.5 * (1.0 - lk[hsl]),
            0.5 * (1.0 + lk[hsl]),
        ]).astype(np.float32)
        x_b_core = np.ascontiguousarray(x[b])
        # MLP-phase tokens: RS rank r gets the contiguous quarter [r*512,(r+1)*512)
        x_res_core = np.ascontiguousarray(x[b][r * MLP_TOK:(r + 1) * MLP_TOK])
        in_maps.append({
            "x_b": x_b_core,
            "x_res": x_res_core,
            "wqkv": wqkv_local,
            "bqkv": bqkv_local,
            "wproj": wproj_local,
            "bproj": b_attn_proj,
            "wfc": wf_b,
            "bfc": bf.astype(np.float32),
            "wmlp": wm_b,
            "bmlp": b_mlp_proj,
            "lif": lif_local,
        })

    global _last_in_maps
    _last_in_maps = in_maps
    nc = _get_nc()
    res = run_bass_kernel_spmd(nc, in_maps, list(range(N_CORES)))

    out = np.empty((B, T, C), np.float32)  # RS shard r is contiguous
    for c in range(N_CORES):
        b = c // GROUP
        r = c % GROUP
        out[b, r * MLP_TOK:(r + 1) * MLP_TOK, :] = res.results[c]["out"]
    return out

